# revision 4
# baseline (speedup 1.0000x reference)
"""BiLSTM-CRF Trainium2 kernel.

Data-parallel over batch: 64 sequences -> 8 cores x 8 sequences.
Per core:
  - embedding gather (indirect DMA), input projections (PE matmuls, fp32)
  - 2-layer BiLSTM recurrence: fwd at partitions 0-7, bwd at 32-39
    (tile_position col-groups), batch-major gates [40, 1024] in PSUM,
    sigmoid/tanh on ScalarE, cell update on VectorE, h transposed back to
    lhsT layout via PE transpose each step
  - FC emission scores, Viterbi forward scan (DVE), parallel backpointer
    extraction, sequential backtrack
Outputs: (score [64,1] f32, path [64,512] int32) matching reference().
"""
import numpy as np
from contextlib import ExitStack

import concourse.bass as bass
import concourse.tile as tile
import concourse.mybir as mybir
from concourse.bass_utils import run_bass_kernel_spmd
from concourse.masks import make_identity
from waitsplit import split_multi_waits

F32 = mybir.dt.float32
I32 = mybir.dt.int32
AF = mybir.ActivationFunctionType
ALU = mybir.AluOpType
AX = mybir.AxisListType

B, T, V, E, H, K = 64, 512, 50002, 300, 256, 11
START, STOP = 9, 10
NCORES = 8
BL = B // NCORES          # 8 sequences per core
NT = BL * T               # 4096 tokens per core
NTILES = NT // 128        # 32 token tiles
G4 = 4 * H                # 1024 gate rows


def _ceil_chunks(n, c=128):
    out = []
    o = 0
    while o < n:
        out.append((o, min(c, n - o)))
        o += c
    return out


def build_kernel(t_steps=T):
    nt = BL * t_steps
    ntiles = nt // 128
    nc = bass.Bass()

    # ---- external inputs (per core) ----
    sent = nc.dram_tensor("sent", [nt, 1], I32, kind="ExternalInput")
    emb = nc.dram_tensor("emb", [V, E], F32, kind="ExternalInput")
    wih0 = [nc.dram_tensor(f"wih0{d}", [E, G4], F32, kind="ExternalInput") for d in "fb"]
    whh0 = [nc.dram_tensor(f"whh0{d}", [H, G4], F32, kind="ExternalInput") for d in "fb"]
    b0 = [nc.dram_tensor(f"b0{d}", [G4], F32, kind="ExternalInput") for d in "fb"]
    wih1 = [nc.dram_tensor(f"wih1{d}", [2 * H, G4], F32, kind="ExternalInput") for d in "fb"]
    whh1 = [nc.dram_tensor(f"whh1{d}", [H, G4], F32, kind="ExternalInput") for d in "fb"]
    b1 = [nc.dram_tensor(f"b1{d}", [G4], F32, kind="ExternalInput") for d in "fb"]
    fcw = nc.dram_tensor("fcw", [2 * H, K], F32, kind="ExternalInput")
    fcb = nc.dram_tensor("fcb", [K], F32, kind="ExternalInput")
    trans_nf = nc.dram_tensor("trans_nf", [K * K], F32, kind="ExternalInput")  # [n*11+p] = trans[n,p]
    trans_stop = nc.dram_tensor("trans_stop", [K], F32, kind="ExternalInput")
    init_fv = nc.dram_tensor("init_fv", [K], F32, kind="ExternalInput")
    inviota_np = nc.dram_tensor("inviota_np", [K * K], F32, kind="ExternalInput")  # 16 - p
    inviota_n = nc.dram_tensor("inviota_n", [K], F32, kind="ExternalInput")        # 16 - n
    iota_n = nc.dram_tensor("iota_n", [K], F32, kind="ExternalInput")              # n

    # ---- outputs ----
    out_score = nc.dram_tensor("out_score", [BL, 1], F32, kind="ExternalOutput")
    out_path = nc.dram_tensor("out_path", [BL, t_steps], I32, kind="ExternalOutput")

    # ---- internal DRAM scratch ----
    xw_d = {(l, d): nc.dram_tensor(f"xw{l}{d}", [nt, G4], F32, kind="Internal")
            for l in (0, 1) for d in (0, 1)}
    h0_d = nc.dram_tensor("h0buf", [nt, 2 * H], F32, kind="Internal")
    h1_d = nc.dram_tensor("h1buf", [nt, 2 * H], F32, kind="Internal")
    feats_d = nc.dram_tensor("featsbuf", [nt, K], F32, kind="Internal")
    fv_d = nc.dram_tensor("fvbuf", [nt, K], F32, kind="Internal")
    bp_d = nc.dram_tensor("bpbuf", [nt, K], F32, kind="Internal")

    _uid = [0]

    def bcast(src_handle, n, width, pool, name=None):
        if name is None:
            name = f"bc{_uid[0]}"
            _uid[0] += 1
        t_ = pool.tile([128, width], F32, name=name, tag=name)
        ap = src_handle[:]
        nc.sync.dma_start(out=t_[:n, :], in_=bass.AP(tensor=ap.tensor, offset=0,
                                                     ap=[[0, n], [1, width]]))
        return t_

    with tile.TileContext(nc) as tc, ExitStack() as octx:
        const = octx.enter_context(tc.tile_pool(name="const", bufs=1))

        ident128 = const.tile([128, 128], F32)
        make_identity(nc, ident128[:, :])
        ident40 = const.tile([40, 40], F32)
        make_identity(nc, ident40[:, :])

        # weights to SBUF
        def load_k(src, rows, width):
            ch = _ceil_chunks(rows)
            t_ = const.tile([128, len(ch), width], F32, name=f"w{_uid[0]}", tag=f"w{_uid[0]}")
            _uid[0] += 1
            for kc, (o, sz) in enumerate(ch):
                nc.sync.dma_start(out=t_[:sz, kc, :], in_=src[o:o + sz, :])
            return t_

        wih0_sb = [load_k(wih0[d], E, G4) for d in range(2)]
        whh0_sb = [load_k(whh0[d], H, G4) for d in range(2)]
        wih1_sb = [load_k(wih1[d], 2 * H, G4) for d in range(2)]
        whh1_sb = [load_k(whh1[d], H, G4) for d in range(2)]
        fcw_sb = load_k(fcw, 2 * H, K)
        bias_sb = {(0, 0): bcast(b0[0], 128, G4, const), (0, 1): bcast(b0[1], 128, G4, const),
                   (1, 0): bcast(b1[0], 128, G4, const), (1, 1): bcast(b1[1], 128, G4, const)}
        fcb_sb = bcast(fcb, 128, K, const)
        trans_b = bcast(trans_nf, 128, K * K, const)
        invnp_b = bcast(inviota_np, 128, K * K, const)
        invn_b = bcast(inviota_n, 8, K, const)
        iota_b = bcast(iota_n, 8, K, const)
        stop_b = bcast(trans_stop, 8, K, const)

        # =========== phase A: gather + layer0 projections ===========
        with tc.tile_pool(name="proj", bufs=3) as pp, \
             tc.tile_pool(name="projps", bufs=2, space="PSUM") as pps, \
             tc.tile_pool(name="trps", bufs=2, space="PSUM") as tps:
            e_ch = _ceil_chunks(E)
            for i in range(ntiles):
                idx_t = pp.tile([128, 1], I32, tag="idx")
                nc.sync.dma_start(out=idx_t[:, :], in_=sent[i * 128:(i + 1) * 128, :])
                xg = pp.tile([128, E], F32, tag="xg")
                nc.gpsimd.indirect_dma_start(
                    out=xg[:, :], out_offset=None, in_=emb[:, :],
                    in_offset=bass.IndirectOffsetOnAxis(ap=idx_t[:, :1], axis=0))
                xt = pp.tile([128, len(e_ch), 128], F32, tag="xt")
                for kc, (o, sz) in enumerate(e_ch):
                    ps = tps.tile([128, 128], F32, tag="tp")
                    nc.tensor.transpose(out=ps[:sz, :], in_=xg[:, o:o + sz],
                                        identity=ident128[:, :])
                    nc.vector.tensor_copy(out=xt[:sz, kc, :], in_=ps[:sz, :])
                for d in range(2):
                    ps = pps.tile([128, G4], F32, tag="xwps")
                    for kc, (o, sz) in enumerate(e_ch):
                        for nh in range(2):
                            nc.tensor.matmul(
                                out=ps[:, nh * 512:(nh + 1) * 512],
                                lhsT=xt[:sz, kc, :],
                                rhs=wih0_sb[d][:sz, kc, nh * 512:(nh + 1) * 512],
                                start=(kc == 0), stop=(kc == len(e_ch) - 1))
                    xwsb = pp.tile([128, G4], F32, tag="xwsb")
                    nc.vector.tensor_add(out=xwsb[:, :], in0=ps[:, :],
                                         in1=bias_sb[(0, d)][:, :])
                    nc.sync.dma_start(out=xw_d[(0, d)][i * 128:(i + 1) * 128, :],
                                      in_=xwsb[:, :])

        # =========== recurrence helper ===========
        def lstm_layer(layer, whh_sb, hout):
            with tc.tile_pool(name=f"rec{layer}", bufs=1) as rp, \
                 tc.tile_pool(name=f"recs{layer}", bufs=2) as rs, \
                 tc.tile_pool(name=f"recps{layer}", bufs=1, space="PSUM") as rps:
                hT = [rp.tile([128, 2, 16], F32, name=f"hT{j}", tag=f"hT{j}") for j in range(2)]
                cst = [rp.tile([40, H], F32, name=f"c{j}", tag=f"c{j}") for j in range(2)]
                gp = [rps.tile([128, G4], F32, name=f"gp{j}", tag=f"gp{j}") for j in range(2)]
                htp = [rps.tile([128, 40], F32, name=f"htp{j}", tag=f"htp{j}") for j in range(2)]
                xwr = [rp.tile([40, G4], F32, name=f"xwr{j}", tag=f"xwr{j}") for j in range(8)]
                for j in range(2):
                    nc.vector.memset(hT[j][:, :, :], 0.0)
                    nc.vector.memset(cst[j][:, :], 0.0)
                    nc.vector.memset(gp[j][:, :], 0.0)
                for j in range(8):
                    nc.vector.memset(xwr[j][:, :], 0.0)

                for s in range(t_steps):
                    tb = t_steps - 1 - s
                    xwt = xwr[s % 8]
                    nc.sync.dma_start(out=xwt[0:8, :],
                                      in_=xw_d[(layer, 0)][s * 8:(s + 1) * 8, :])
                    nc.sync.dma_start(out=xwt[32:40, :],
                                      in_=xw_d[(layer, 1)][tb * 8:(tb + 1) * 8, :])
                    g = gp[s % 2]
                    hin = hT[s % 2]
                    hnew = hT[(s + 1) % 2]
                    cin = cst[s % 2]
                    cnew = cst[(s + 1) % 2]
                    for d, (pb, tp, cols) in enumerate(
                            [(0, (0, 0), (0, 8)), (32, (0, 32), (8, 16))]):
                        for nh in range(2):
                            for kc in range(2):
                                nc.tensor.matmul(
                                    out=g[pb:pb + 8, nh * 512:(nh + 1) * 512],
                                    lhsT=hin[:, kc, cols[0]:cols[1]],
                                    rhs=whh_sb[d][:, kc, nh * 512:(nh + 1) * 512],
                                    start=(kc == 0), stop=(kc == 1),
                                    tile_position=tp)
                    nc.vector.tensor_add(out=g[0:40, :], in0=g[0:40, :],
                                         in1=xwt[0:40, :])
                    sig = rs.tile([40, 768], F32, tag="sig")
                    tg = rs.tile([40, H], F32, tag="tg")
                    nc.scalar.activation(out=sig[:, :], in_=g[0:40, 0:768], func=AF.Sigmoid)
                    nc.scalar.activation(out=tg[:, :], in_=g[0:40, 768:1024], func=AF.Tanh)
                    t1 = rs.tile([40, H], F32, tag="t1")
                    t2 = rs.tile([40, H], F32, tag="t2")
                    nc.vector.tensor_mul(out=t1[:, :], in0=sig[:, 256:512], in1=cin[:, :])
                    nc.vector.tensor_mul(out=t2[:, :], in0=sig[:, 0:256], in1=tg[:, :])
                    nc.vector.tensor_add(out=cnew[:, :], in0=t1[:, :], in1=t2[:, :])
                    tc_ = rs.tile([40, H], F32, tag="tc")
                    nc.scalar.activation(out=tc_[:, :], in_=cnew[:, :], func=AF.Tanh)
                    hsb = rs.tile([40, H], F32, tag="hsb")
                    nc.vector.tensor_mul(out=hsb[:, :], in0=sig[:, 512:768], in1=tc_[:, :])
                    for kc in range(2):
                        hp = htp[(2 * s + kc) % 2]
                        nc.tensor.transpose(out=hp[:, :], in_=hsb[:, kc * 128:(kc + 1) * 128],
                                            identity=ident40[:, :])
                        hpa = hp[:, :]
                        sel = bass.AP(tensor=hpa.tensor, offset=hpa.offset,
                                      ap=[hpa.ap[0], [32, 2], [1, 8]])
                        nc.vector.tensor_copy(out=hnew[:, kc, 0:16], in_=sel)
                    nc.sync.dma_start(out=hout[s * 8:(s + 1) * 8, 0:H], in_=hsb[0:8, :])
                    nc.sync.dma_start(out=hout[tb * 8:(tb + 1) * 8, H:2 * H],
                                      in_=hsb[32:40, :])

        # =========== layer 0 recurrence ===========
        lstm_layer(0, whh0_sb, h0_d)

        # =========== layer 1 projections ===========
        with tc.tile_pool(name="proj1", bufs=3) as pp, \
             tc.tile_pool(name="proj1ps", bufs=2, space="PSUM") as pps, \
             tc.tile_pool(name="tr1ps", bufs=2, space="PSUM") as tps:
            h_ch = _ceil_chunks(2 * H)
            for i in range(ntiles):
                hg = pp.tile([128, 2 * H], F32, tag="hg")
                nc.sync.dma_start(out=hg[:, :], in_=h0_d[i * 128:(i + 1) * 128, :])
                ht_ = pp.tile([128, len(h_ch), 128], F32, tag="ht")
                for kc, (o, sz) in enumerate(h_ch):
                    ps = tps.tile([128, 128], F32, tag="tp1")
                    nc.tensor.transpose(out=ps[:sz, :], in_=hg[:, o:o + sz],
                                        identity=ident128[:, :])
                    nc.vector.tensor_copy(out=ht_[:sz, kc, :], in_=ps[:sz, :])
                for d in range(2):
                    ps = pps.tile([128, G4], F32, tag="xw1ps")
                    for kc, (o, sz) in enumerate(h_ch):
                        for nh in range(2):
                            nc.tensor.matmul(
                                out=ps[:, nh * 512:(nh + 1) * 512],
                                lhsT=ht_[:sz, kc, :],
                                rhs=wih1_sb[d][:sz, kc, nh * 512:(nh + 1) * 512],
                                start=(kc == 0), stop=(kc == len(h_ch) - 1))
                    xwsb = pp.tile([128, G4], F32, tag="xw1sb")
                    nc.vector.tensor_add(out=xwsb[:, :], in0=ps[:, :],
                                         in1=bias_sb[(1, d)][:, :])
                    nc.sync.dma_start(out=xw_d[(1, d)][i * 128:(i + 1) * 128, :],
                                      in_=xwsb[:, :])

        # =========== layer 1 recurrence ===========
        lstm_layer(1, whh1_sb, h1_d)

        # =========== FC -> feats (token-major) ===========
        with tc.tile_pool(name="fc", bufs=3) as pp, \
             tc.tile_pool(name="fcps", bufs=2, space="PSUM") as pps, \
             tc.tile_pool(name="fctps", bufs=2, space="PSUM") as tps:
            h_ch = _ceil_chunks(2 * H)
            for i in range(ntiles):
                hg = pp.tile([128, 2 * H], F32, tag="h1g")
                nc.sync.dma_start(out=hg[:, :], in_=h1_d[i * 128:(i + 1) * 128, :])
                ht_ = pp.tile([128, len(h_ch), 128], F32, tag="h1t")
                for kc, (o, sz) in enumerate(h_ch):
                    ps = tps.tile([128, 128], F32, tag="tpf")
                    nc.tensor.transpose(out=ps[:sz, :], in_=hg[:, o:o + sz],
                                        identity=ident128[:, :])
                    nc.vector.tensor_copy(out=ht_[:sz, kc, :], in_=ps[:sz, :])
                ps = pps.tile([128, K], F32, tag="fcp")
                for kc, (o, sz) in enumerate(h_ch):
                    nc.tensor.matmul(out=ps[:, :], lhsT=ht_[:sz, kc, :],
                                     rhs=fcw_sb[:sz, kc, :],
                                     start=(kc == 0), stop=(kc == len(h_ch) - 1))
                fsb = pp.tile([128, K], F32, tag="fsb")
                nc.vector.tensor_add(out=fsb[:, :], in0=ps[:, :], in1=fcb_sb[:, :])
                nc.sync.dma_start(out=feats_d[i * 128:(i + 1) * 128, :], in_=fsb[:, :])

        # =========== Viterbi ===========
        with tc.tile_pool(name="vit", bufs=1) as vp, \
             tc.tile_pool(name="vits", bufs=2) as vs:
            # feats batch-major [8, t, K]
            feats_bm = vp.tile([8, t_steps, K], F32)
            fap = feats_d[:, :]
            fbm = feats_bm[:, :, :]
            for j in range(16):
                srcap = bass.AP(tensor=fap.tensor, offset=j * 8 * K,
                                ap=[[K, 8], [16 * 8 * K, t_steps // 16], [1, K]])
                dstap = bass.AP(tensor=fbm.tensor, offset=fbm.offset + j * K,
                                ap=[fbm.ap[0], [16 * K, t_steps // 16], [1, K]])
                nc.sync.dma_start(out=dstap, in_=srcap)
            # forward scan; fv_hist slot t = fv_{t-1}; slot 0 = init
            fv_hist = vp.tile([8, t_steps, K], F32)
            iap = init_fv[:]
            nc.sync.dma_start(out=fv_hist[0:8, 0, :],
                              in_=bass.AP(tensor=iap.tensor, offset=0,
                                          ap=[[0, 8], [1, K]]))
            fv_last = vp.tile([8, K], F32)
            for t in range(t_steps):
                prev = fv_hist[:, t, :]
                pap = prev
                prev_b = bass.AP(tensor=pap.tensor, offset=pap.offset,
                                 ap=[pap.ap[0], [0, K], [1, K]])
                sc = vs.tile([8, K * K], F32, tag="sc")
                nc.vector.tensor_add(out=sc[:, :], in0=prev_b, in1=trans_b[0:8, :])
                vit = vs.tile([8, K], F32, tag="vit")
                nc.vector.tensor_reduce(out=vit[:, :],
                                        in_=sc[:, :].rearrange("p (a b) -> p a b", a=K),
                                        axis=AX.X, op=ALU.max)
                dst = fv_hist[:, t + 1, :] if t < t_steps - 1 else fv_last[:, :]
                nc.vector.tensor_add(out=dst, in0=vit[:, :], in1=feats_bm[:, t, :])
            # fv_hist -> DRAM (token-major rows t*8+b hold fv_{t-1})
            fvap = fv_d[:, :]
            fhm = fv_hist[:, :, :]
            for j in range(16):
                dst = bass.AP(tensor=fvap.tensor, offset=j * 8 * K,
                              ap=[[K, 8], [16 * 8 * K, t_steps // 16], [1, K]])
                srcap = bass.AP(tensor=fhm.tensor, offset=fhm.offset + j * K,
                                ap=[fhm.ap[0], [16 * K, t_steps // 16], [1, K]])
                nc.sync.dma_start(out=dst, in_=srcap)
            # terminal score + last tag
            term = vp.tile([8, K], F32)
            nc.vector.tensor_add(out=term[:, :], in0=fv_last[:, :], in1=stop_b[0:8, :])
            score_sb = vp.tile([8, 1], F32)
            nc.vector.tensor_reduce(out=score_sb[:, :], in_=term[:, :], axis=AX.X,
                                    op=ALU.max)
            nc.sync.dma_start(out=out_score[:, :], in_=score_sb[:, :])
            path_f = vp.tile([8, t_steps], F32)
            eq = vp.tile([8, K], F32)
            nc.vector.tensor_scalar(out=eq[:, :], in0=term[:, :],
                                    scalar1=score_sb[:, :1], scalar2=None,
                                    op0=ALU.is_equal)
            u8 = vp.tile([8, K], F32)
            nc.vector.tensor_mul(out=u8[:, :], in0=eq[:, :], in1=invn_b[0:8, :])
            w8 = vp.tile([8, 1], F32)
            nc.vector.tensor_reduce(out=w8[:, :], in_=u8[:, :], axis=AX.X, op=ALU.max)
            nc.vector.tensor_scalar(out=path_f[:, t_steps - 1:t_steps], in0=w8[:, :],
                                    scalar1=16.0, scalar2=-1.0,
                                    op0=ALU.subtract, op1=ALU.mult)

        # backpointer extraction (token-parallel)
        with tc.tile_pool(name="bpx", bufs=3) as bpp:
            for i in range(ntiles):
                fvt = bpp.tile([128, K], F32, tag="fvt")
                nc.sync.dma_start(out=fvt[:, :], in_=fv_d[i * 128:(i + 1) * 128, :])
                fap2 = fvt[:, :]
                fv_b = bass.AP(tensor=fap2.tensor, offset=fap2.offset,
                               ap=[fap2.ap[0], [0, K], [1, K]])
                sc = bpp.tile([128, K * K], F32, tag="bsc")
                nc.vector.tensor_add(out=sc[:, :], in0=fv_b, in1=trans_b[:, :])
                vit = bpp.tile([128, K], F32, tag="bvit")
                nc.vector.tensor_reduce(out=vit[:, :],
                                        in_=sc[:, :].rearrange("p (a b) -> p a b", a=K),
                                        axis=AX.X, op=ALU.max)
                vap = vit[:, :]
                vit_b = bass.AP(tensor=vap.tensor, offset=vap.offset,
                                ap=[vap.ap[0], [1, K], [0, K]])
                eqt = bpp.tile([128, K * K], F32, tag="beq")
                nc.vector.tensor_tensor(out=eqt[:, :], in0=sc[:, :], in1=vit_b,
                                        op=ALU.is_equal)
                ut = bpp.tile([128, K * K], F32, tag="bu")
                nc.vector.tensor_mul(out=ut[:, :], in0=eqt[:, :], in1=invnp_b[:, :])
                wt = bpp.tile([128, K], F32, tag="bw")
                nc.vector.tensor_reduce(out=wt[:, :],
                                        in_=ut[:, :].rearrange("p (a b) -> p a b", a=K),
                                        axis=AX.X, op=ALU.max)
                bpt = bpp.tile([128, K], F32, tag="bbp")
                nc.vector.tensor_scalar(out=bpt[:, :], in0=wt[:, :],
                                        scalar1=16.0, scalar2=-1.0,
                                        op0=ALU.subtract, op1=ALU.mult)
                nc.sync.dma_start(out=bp_d[i * 128:(i + 1) * 128, :], in_=bpt[:, :])

        # backtrack (sequential)
        with tc.tile_pool(name="bt", bufs=1) as btp, \
             tc.tile_pool(name="bts", bufs=2) as bts:
            bp_bm = btp.tile([8, t_steps, K], F32)
            bap = bp_d[:, :]
            bbm = bp_bm[:, :, :]
            for j in range(16):
                srcap = bass.AP(tensor=bap.tensor, offset=j * 8 * K,
                                ap=[[K, 8], [16 * 8 * K, t_steps // 16], [1, K]])
                dstap = bass.AP(tensor=bbm.tensor, offset=bbm.offset + j * K,
                                ap=[bbm.ap[0], [16 * K, t_steps // 16], [1, K]])
                nc.sync.dma_start(out=dstap, in_=srcap)
            for t in range(t_steps - 1, 0, -1):
                m = bts.tile([8, K], F32, tag="btm")
                nc.vector.tensor_scalar(out=m[:, :], in0=iota_b[0:8, :],
                                        scalar1=path_f[:, t:t + 1], scalar2=None,
                                        op0=ALU.is_equal)
                pm = bts.tile([8, K], F32, tag="btp")
                nc.vector.tensor_mul(out=pm[:, :], in0=m[:, :], in1=bp_bm[:, t, :])
                nc.vector.tensor_reduce(out=path_f[:, t - 1:t], in_=pm[:, :],
                                        axis=AX.X, op=ALU.add)
            path_i = btp.tile([8, t_steps], I32)
            nc.vector.tensor_copy(out=path_i[:, :], in_=path_f[:, :])
            nc.sync.dma_start(out=out_path[:, :], in_=path_i[:, :])

    split_multi_waits(nc)
    return nc


# ---------------- host side ----------------

_GATE_PERM = None


def _gate_perm():
    global _GATE_PERM
    if _GATE_PERM is None:
        # reorder i,f,g,o -> i,f,o,g
        _GATE_PERM = np.concatenate([np.arange(0, 2 * H), np.arange(3 * H, 4 * H),
                                     np.arange(2 * H, 3 * H)])
    return _GATE_PERM


def _prep_weights(inputs):
    p = _gate_perm()
    d = {}
    for l, lname in ((0, "l0"), (1, "l1")):
        for di, dn in ((0, "f"), (1, "b")):
            wih = np.asarray(inputs[f"w_ih_{lname}{dn}"])[p]      # [1024, Din]
            whh = np.asarray(inputs[f"w_hh_{lname}{dn}"])[p]      # [1024, 256]
            bb = np.asarray(inputs[f"b_{lname}{dn}"])[p]          # [1024]
            d[f"wih{l}{'fb'[di]}"] = np.ascontiguousarray(wih.T.astype(np.float32))
            d[f"whh{l}{'fb'[di]}"] = np.ascontiguousarray(whh.T.astype(np.float32))
            d[f"b{l}{'fb'[di]}"] = np.ascontiguousarray(bb.astype(np.float32))
    d["fcw"] = np.ascontiguousarray(np.asarray(inputs["fc_w"]).T.astype(np.float32))
    d["fcb"] = np.ascontiguousarray(np.asarray(inputs["fc_b"]).astype(np.float32))
    trans = np.asarray(inputs["transitions"]).astype(np.float32)
    d["trans_nf"] = np.ascontiguousarray(trans.reshape(-1))
    d["trans_stop"] = np.ascontiguousarray(trans[STOP])
    init = np.full(K, -1000.0, np.float32)
    init[START] = 0.0
    d["init_fv"] = init
    d["inviota_np"] = np.tile(16.0 - np.arange(K, dtype=np.float32), K)
    d["inviota_n"] = 16.0 - np.arange(K, dtype=np.float32)
    d["iota_n"] = np.arange(K, dtype=np.float32)
    d["emb"] = np.ascontiguousarray(np.asarray(inputs["emb"]).astype(np.float32))
    return d


_NC_CACHE = {}


def kernel(**inputs):
    t_steps = np.asarray(inputs["sentence"]).shape[1]
    if t_steps not in _NC_CACHE:
        _NC_CACHE[t_steps] = build_kernel(t_steps)
    nc = _NC_CACHE[t_steps]

    shared = _prep_weights(inputs)
    sentence = np.asarray(inputs["sentence"])
    in_maps = []
    for c in range(NCORES):
        sl = sentence[c * BL:(c + 1) * BL]              # [8, T]
        sent_tm = np.ascontiguousarray(sl.T.reshape(-1, 1).astype(np.int32))
        m = dict(shared)
        m["sent"] = sent_tm
        in_maps.append(m)

    res = run_bass_kernel_spmd(nc, in_maps, core_ids=list(range(NCORES)))
    scores = np.concatenate([r["out_score"] for r in res.results], 0)
    paths = np.concatenate([r["out_path"] for r in res.results], 0)
    return scores.astype(np.float32), paths.astype(np.int32)
